# revision 7
# baseline (speedup 1.0000x reference)
"""DriftAwareMultiHeadAttention on 8 Trainium2 NeuronCores.

Sharding: core c -> (batch b = c//2, head-group hg = c%2).
Each core runs QKV projection (column-parallel over its 8 heads), full
attention for those heads, and a partial output projection
(row-parallel).  Host gathers: y[b] = (yT[2b] + yT[2b+1]).T + b_out.

Device layout is feature-on-partition / token-on-free throughout, so no
transposes are ever needed on chip:
  - Q^T, K^T: [512, 2048]  (head h -> e-tile h//2, partition offset (h%2)*64)
  - V:        [tokens, 8 heads x (64+1)]  -- the extra "ones" column makes
              the attention matmul emit the softmax denominator for free.
  - scores S^T = K^T-tile @ Q (k on partitions, q on free) -> exp via
    ScalarE activation (scale folded in), AV accumulates [65, 512] in PSUM,
    normalization via DVE reciprocal + ones-matmul partition-broadcast.
Matmuls run in float32r (full PE speed for free-dim >= 256).
"""

import math

import numpy as np

import concourse.bass as bass
import concourse.mybir as mybir
import concourse.tile as tile
from concourse import bacc
from concourse.bass import ds, ts
from concourse.bass_utils import run_bass_kernel_spmd

P = 128
T = 2048        # tokens per batch
DM = 1024       # model dim
E = 512         # per-core projection width (8 heads * 64)
H = 8           # heads per core
HD = 64
CD = DM // P    # contraction chunks over model dim
NKT = T // P    # k tiles per head
QC = 512        # q chunk
NQC = T // QC
F32 = mybir.dt.float32
F32R = mybir.dt.float32r
BF16 = mybir.dt.bfloat16
EXP = mybir.ActivationFunctionType.Exp


def _r(ap):
    return ap.bitcast(F32R)


def build(scale: float, use_bf16: bool = False):
    MDT = BF16 if use_bf16 else F32R
    nc = bacc.Bacc(None, target_bir_lowering=False, debug=False)
    xT = nc.declare_dram_parameter("xT", [DM, T], MDT, isOutput=False)
    wq = nc.declare_dram_parameter("wq", [DM, E], MDT, isOutput=False)
    wk = nc.declare_dram_parameter("wk", [DM, E], MDT, isOutput=False)
    wv = nc.declare_dram_parameter("wv", [DM, E], MDT, isOutput=False)
    wo = nc.declare_dram_parameter("wo", [E, DM], F32, isOutput=False)
    yT = nc.declare_dram_parameter("yT", [DM, T], F32, isOutput=True)

    with tile.TileContext(nc) as tc:
        with (
            tc.tile_pool(name="qk", bufs=1) as qkp,
            tc.tile_pool(name="vp", bufs=1) as vp,
            tc.tile_pool(name="misc", bufs=1) as miscp,
        ):
            QT = qkp.tile([P, 4, T], MDT, tag="QT")
            KT = qkp.tile([P, 4, T], MDT, tag="KT")
            V = vp.tile([P, NKT, H, HD + 1], MDT, tag="V")
            nc.vector.memset(V[:, :, :, HD : HD + 1].bitcast(F32) if not use_bf16 else V[:, :, :, HD : HD + 1], 1.0)

            # ---------------- phase 1: projections ----------------
            with (
                tc.tile_pool(name="wts", bufs=1) as wp,
                tc.tile_pool(name="xt", bufs=1) as xp,
                tc.tile_pool(name="p1", bufs=2, space="PSUM") as p1pool,
            ):
                wq_sb = wp.tile([P, CD, E], MDT, tag="wq")
                wk_sb = wp.tile([P, CD, E], MDT, tag="wk")
                wv_sb = wp.tile([P, CD, E], MDT, tag="wv")
                nc.sync.dma_start(out=wq_sb[:], in_=wq.rearrange("(c p) e -> p c e", p=P))
                nc.sync.dma_start(out=wk_sb[:], in_=wk.rearrange("(c p) e -> p c e", p=P))
                nc.sync.dma_start(out=wv_sb[:], in_=wv.rearrange("(c p) e -> p c e", p=P))
                TH = T // 2
                for th in range(2):
                    xts = []
                    for c in range(CD):
                        xt = xp.tile([P, TH], MDT, tag=f"x{c}")
                        nc.sync.dma_start(
                            out=xt[:], in_=xT[c * P : (c + 1) * P, th * TH : (th + 1) * TH]
                        )
                        xts.append(xt)
                    for wsb, dst in ((wq_sb, QT), (wk_sb, KT)):
                        for et in range(4):
                            for tcl in range(2):
                                ps = p1pool.tile([P, QC], F32, tag="pp")
                                for c in range(CD):
                                    nc.tensor.matmul(
                                        ps[:],
                                        wsb[:, c, ts(et, P)],
                                        xts[c][:, ts(tcl, QC)],
                                        start=(c == 0),
                                        stop=(c == CD - 1),
                                    )
                                nc.vector.tensor_copy(
                                    dst[:, et, ds(th * TH + tcl * QC, QC)], ps[:]
                                )
                    for ttl in range(TH // P):
                        tt = th * (TH // P) + ttl
                        ps = p1pool.tile([P, E], F32, tag="pp")
                        for c in range(CD):
                            nc.tensor.matmul(
                                ps[:],
                                xts[c][:, ts(ttl, P)],
                                wv_sb[:, c, :],
                                start=(c == 0),
                                stop=(c == CD - 1),
                            )
                        nc.vector.tensor_copy(
                            V[:, tt, :, 0:HD],
                            ps[:].rearrange("p (h e) -> p h e", h=H),
                        )

            # ---------------- phases 2+3 ----------------
            with tc.tile_pool(name="outp", bufs=1) as outp:
                outT = outp.tile([P, 4, T], F32R, tag="outT")

                with (
                    tc.tile_pool(name="pbuf", bufs=3) as pbuf,
                    tc.tile_pool(name="nrm", bufs=2) as nrmp,
                    tc.tile_pool(name="sps", bufs=3, space="PSUM") as spool,
                    tc.tile_pool(name="ovp", bufs=2, space="PSUM") as ovpool,
                ):

                    def emit_scores_half(h, qc, half):
                        et, off = h // 2, (h % 2) * HD
                        Ph = pbuf.tile([P, 8, QC], MDT, tag="P")
                        for j in range(4):
                            sp = spool.tile([P, 2 * QC], F32, tag="S")
                            for u in range(2):
                                kt = half * 8 + j * 2 + u
                                nc.tensor.matmul(
                                    sp[:, u * QC : (u + 1) * QC],
                                    KT[off : off + HD, et, kt * P : (kt + 1) * P],
                                    QT[off : off + HD, et, ts(qc, QC)],
                                    start=True,
                                    stop=True,
                                )
                            nc.scalar.activation(
                                out=Ph[:, 2 * j : 2 * j + 2, :],
                                in_=sp[:].rearrange("p (a b) -> p a b", b=QC),
                                func=EXP,
                                scale=scale,
                            )
                        return Ph

                    def emit_av_round(h, opsum, Ph, rnd):
                        for jl in range(8):
                            kt = rnd * 8 + jl
                            nc.tensor.matmul(
                                opsum[0 : HD + 1, :],
                                V[:, kt, h, :],
                                Ph[:, jl, :],
                                start=(kt == 0),
                                stop=(kt == NKT - 1),
                            )

                    def emit_finish(pv):
                        opsum, recip, h, qc = pv
                        et, off = h // 2, (h % 2) * HD
                        bcs = nrmp.tile([HD, QC], F32, tag="bcs")
                        nc.gpsimd.partition_broadcast(bcs[:], recip[:], channels=HD)
                        nc.vector.tensor_mul(
                            outT[off : off + HD, et, ts(qc, QC)],
                            opsum[0:HD, :],
                            bcs[:],
                        )

                    units = [(h, qc) for h in range(H) for qc in range(NQC)]
                    prev = None  # (opsum, P1, recip-pending state)
                    for h, qc in units:
                        P0 = emit_scores_half(h, qc, 0)
                        if prev is not None:
                            popsum, pP1, ph, pqc = prev
                            emit_av_round(ph, popsum, pP1, 1)
                            den = nrmp.tile([1, QC], F32, tag="dn")
                            nc.vector.tensor_copy(den[:], popsum[HD : HD + 1, :])
                            recip = nrmp.tile([1, QC], F32, tag="rc")
                            nc.vector.reciprocal_approx_fast(recip[:], den[:])
                        P1 = emit_scores_half(h, qc, 1)
                        if prev is not None:
                            emit_finish((popsum, recip, ph, pqc))
                        opsum = ovpool.tile([P, QC], F32, tag="ov")
                        emit_av_round(h, opsum, P0, 0)
                        prev = (opsum, P1, h, qc)
                    # drain the pipeline tail
                    popsum, pP1, ph, pqc = prev
                    emit_av_round(ph, popsum, pP1, 1)
                    den = nrmp.tile([1, QC], F32, tag="dn")
                    nc.vector.tensor_copy(den[:], popsum[HD : HD + 1, :])
                    recip = nrmp.tile([1, QC], F32, tag="rc")
                    nc.vector.reciprocal_approx_fast(recip[:], den[:])
                    emit_finish((popsum, recip, ph, pqc))

                # ---------------- phase 3: output projection ----------------
                with (
                    tc.tile_pool(name="wop", bufs=1) as wop,
                    tc.tile_pool(name="yev", bufs=3) as yev,
                    tc.tile_pool(name="yps", bufs=3, space="PSUM") as ypool,
                ):
                    wo_sb = wop.tile([P, 4, DM], F32R, tag="wo")
                    nc.sync.dma_start(
                        out=wo_sb[:], in_=wo.bitcast(F32R).rearrange("(c p) e -> p c e", p=P)
                    )
                    for et in range(DM // P):
                        for tcq in range(NQC):
                            ps = ypool.tile([P, QC], F32, tag="y")
                            for fc in range(4):
                                nc.tensor.matmul(
                                    ps[:],
                                    wo_sb[:, fc, ts(et, P)],
                                    outT[:, fc, ts(tcq, QC)],
                                    start=(fc == 0),
                                    stop=(fc == 3),
                                )
                            yt = yev.tile([P, QC], F32, tag="ye")
                            nc.vector.tensor_copy(yt[:], ps[:])
                            nc.sync.dma_start(
                                out=yT[et * P : (et + 1) * P, ts(tcq, QC)], in_=yt[:]
                            )

    nc.compile()
    return nc


_CACHE: dict = {}


def _get_program(scale: float):
    key = (round(float(scale), 12), USE_BF16)
    if key not in _CACHE:
        _CACHE[key] = build(key[0], USE_BF16)
    return _CACHE[key]


def _make_in_maps(x, w_qkv, w_out, use_bf16):
    import ml_dtypes

    cdt = ml_dtypes.bfloat16 if use_bf16 else np.float32
    xTs = [np.ascontiguousarray(x[b].T).astype(cdt) for b in range(4)]
    wslices = []
    for hg in range(2):
        sl = slice(hg * E, (hg + 1) * E)
        wslices.append(
            {
                "wq": np.ascontiguousarray(w_qkv[0 * DM :][sl, :].T).astype(cdt),
                "wk": np.ascontiguousarray(w_qkv[1 * DM :][sl, :].T).astype(cdt),
                "wv": np.ascontiguousarray(w_qkv[2 * DM :][sl, :].T).astype(cdt),
                "wo": np.ascontiguousarray(w_out[:, sl].T),
            }
        )
    in_maps = []
    for c in range(8):
        b, hg = c // 2, c % 2
        m = {"xT": xTs[b]}
        m.update(wslices[hg])
        in_maps.append(m)
    return in_maps


USE_BF16 = False


def _execute(x, w_qkv, w_out, rescale, **spmd_kwargs):
    scale = float(np.asarray(rescale)) / math.sqrt(HD)
    nc = _get_program(scale)
    in_maps = _make_in_maps(x, w_qkv, w_out, USE_BF16)
    return run_bass_kernel_spmd(nc, in_maps, list(range(8)), **spmd_kwargs)


def kernel(x, w_qkv, w_out, b_out, rescale):
    x = np.asarray(x, dtype=np.float32)
    w_qkv = np.asarray(w_qkv, dtype=np.float32)
    w_out = np.asarray(w_out, dtype=np.float32)
    b_out = np.asarray(b_out, dtype=np.float32)
    res = _execute(x, w_qkv, w_out, rescale).results
    y = np.empty((4, T, DM), dtype=np.float32)
    for b in range(4):
        acc = res[2 * b]["yT"] + res[2 * b + 1]["yT"]
        y[b] = acc.T + b_out
    return y
